# revision 2
# baseline (speedup 1.0000x reference)
"""Trainium2 Bass kernel for CARE position encoding (rotor sandwich product).

The reference computes out = R x R~ where R is a product of 4 plane rotors
exp-like (cos(phi_i) + sin(phi_i) e_mi) with phi_i = 0.5 * c_i * theta[pos, i].
Algebraically this factorizes into 4 sequential Givens-rotation stages: for
plane bivector e_m, the 8 basis blades A with |A & m| == 1 rotate in 4
disjoint pairs (A, A^m) by angle 2*phi, with pair signs tau = C[A, m, A^m];
the other 8 blades pass through unchanged.

Device pipeline per position (data-parallel across 8 cores, batch-sharded):
  th_i = float(pos) * f_i            (f_i = theta table row 1 = frequencies)
  A_i  = th_i * c_i                  (bit-exact match of reference's 2*phi)
  r_i  = A_i mod 2pi                 (round + Cody-Waite cascade)
  s2   = sin(r), c2 = sin(wrap(r + pi/2))
  4x Givens stages on the 16 multivector components.
"""
import numpy as np

import concourse.bass as bass
import concourse.tile as tile
from concourse import bacc, mybir
from concourse.bass_utils import run_bass_kernel_spmd

F32 = mybir.dt.float32
I32 = mybir.dt.int32
ALU = mybir.AluOpType

P = 128
NCORES = 8
B, L, MV = 16, 16384, 16
MAX_LEN = 16384
ROWS_PER_CORE = B // NCORES          # 2
N = ROWS_PER_CORE * L                # 32768 positions per core
J = N // P                           # 256 positions per partition
JT = 64                              # positions per partition per tile
NT = J // JT

PLANE_BLADES = (3, 5, 9, 6)

MAGIC = float(np.float32(1.5 * 2 ** 23))
TWO_PI = 2.0 * np.pi
INV_2PI = float(np.float32(1.0 / TWO_PI))
PI_F = float(np.float32(np.pi))
HALF_PI = float(np.float32(np.pi / 2.0))
TWO_PI_F = float(np.float32(TWO_PI))


def _cw_split(val, bits=12):
    """3-term Cody-Waite split of val: c1 + c2 + c3, c1/c2 with `bits` sig bits."""
    def trunc(v):
        u = np.float32(v).view(np.uint32)
        u = np.uint32(u & np.uint32((0xFFFFFFFF << (24 - bits)) & 0xFFFFFFFF))
        return u.view(np.float32)
    c1 = trunc(val)
    c2 = trunc(val - np.float64(c1))
    c3 = np.float32(val - np.float64(c1) - np.float64(c2))
    return float(c1), float(c2), float(c3)


CW1, CW2, CW3 = _cw_split(TWO_PI)

# Component-pair structure per plane, on the X tile viewed as
# [P, j, h, s, k, r] with comp = 8h + 4s + 2k + r.  For each plane:
# `a`/`b` index the (h, s, k, r) dims to select the 4 pair-halves (AP shape
# [P, JT, 2, 2]); the tau sign pattern over the two remaining free dims is
# affine: tau = (-1)^(base + sig2*d2 + sig3*d3), realized by strided reads of
# an [s2 | -s2 | s2 | -s2] slot tile.  `adims` names which comp dims remain.
_PLANE_GEOM = {
    3: dict(a=(slice(None), slice(None), 0, 1), b=(slice(None), slice(None), 1, 0),
            adims=("h", "s")),
    5: dict(a=(slice(None), 0, slice(None), 1), b=(slice(None), 1, slice(None), 0),
            adims=("h", "k")),
    9: dict(a=(0, slice(None), slice(None), 1), b=(1, slice(None), slice(None), 0),
            adims=("s", "k")),
    6: dict(a=(slice(None), 0, 1, slice(None)), b=(slice(None), 1, 0, slice(None)),
            adims=("h", "r")),
}

_DIM_BIT = {"h": 3, "s": 2, "k": 1, "r": 0}


def _derive_tau_pattern(cayley, m, geom):
    """From the input Cayley tensor, fit tau(A) = (-1)^(base + s2*d2 + s3*d3)
    over the plane's a-set.  Returns (base, sig2, sig3)."""
    d2n, d3n = geom["adims"]
    vals = {}
    for d2 in range(2):
        for d3 in range(2):
            coords = dict(h=0, s=0, k=0, r=0)
            sel = geom["a"]
            names = ("h", "s", "k", "r")
            for nme, sl in zip(names, sel):
                if isinstance(sl, int):
                    coords[nme] = sl
            coords[d2n] = d2
            coords[d3n] = d3
            a = 8 * coords["h"] + 4 * coords["s"] + 2 * coords["k"] + coords["r"]
            tau = float(cayley[a, m, a ^ m])
            assert tau in (1.0, -1.0), f"unexpected cayley entry for blade {a}, plane {m}"
            vals[(d2, d3)] = 0 if tau > 0 else 1
    base = vals[(0, 0)]
    sig2 = vals[(1, 0)] ^ base
    sig3 = vals[(0, 1)] ^ base
    assert vals[(1, 1)] == base ^ sig2 ^ sig3, \
        f"tau pattern for plane {m} is not affine: {vals}"
    return base, sig2, sig3


def _slot_ap(ss, base_off, jt, sig2, sig3, D):
    """AP of shape [P, jt, 2, 2] into the slot tile SS ([s2|-s2|s2|-s2] blocks
    of width D): element (p, j, d2, d3) reads SS[p, base_off + j + (sig2*d2 +
    sig3*d3) * D]."""
    a = ss[:, base_off:base_off + jt].unsqueeze(2).unsqueeze(3).broadcast_to(
        [P, jt, 2, 2])
    ap = [list(d) for d in a.ap]
    ap[2][0] = sig2 * D
    ap[3][0] = sig3 * D
    return bass.AP(a.tensor, a.offset, ap)


def _build_program(freqs, coefs, taus):
    """Trace the Bass/Tile program.  freqs/coefs: 4 floats each; taus: per
    plane (base, sig2, sig3)."""
    nc = bacc.Bacc("TRN2", target_bir_lowering=False, debug=False,
                   enable_asserts=False, num_devices=NCORES)
    x_d = nc.dram_tensor("x", [P, J * MV], F32, kind="ExternalInput")
    pos_d = nc.dram_tensor("pos", [P, J], I32, kind="ExternalInput")
    out_d = nc.dram_tensor("out", [P, J * MV], F32, kind="ExternalOutput")
    D = 4 * JT

    SIN = mybir.ActivationFunctionType.Sin

    with tile.TileContext(nc) as tc:
        with tc.tile_pool(name="x", bufs=3) as xpool, \
             tc.tile_pool(name="pos", bufs=2) as ppool, \
             tc.tile_pool(name="ang", bufs=2) as apool, \
             tc.tile_pool(name="tmp", bufs=2) as tpool:
            for t in range(NT):
                X = xpool.tile([P, JT * MV], F32)
                nc.sync.dma_start(X[:], x_d[:, t * JT * MV:(t + 1) * JT * MV])
                Pp = ppool.tile([P, JT], I32)
                nc.sync.dma_start(Pp[:], pos_d[:, t * JT:(t + 1) * JT])

                posf = apool.tile([P, JT], F32)
                nc.vector.tensor_copy(posf[:], Pp[:])

                # Unreduced angles A_i = (posf * f_i) * c_i, one JT block per
                # plane.  This reproduces the reference's effective rotation
                # angle 2*phi bit-exactly.
                TH = apool.tile([P, 4 * JT], F32)
                A = apool.tile([P, 4 * JT], F32)
                for i in range(4):
                    nc.vector.tensor_scalar_mul(
                        TH[:, i * JT:(i + 1) * JT], posf[:], float(freqs[i]))
                    nc.vector.tensor_scalar_mul(
                        A[:, i * JT:(i + 1) * JT],
                        TH[:, i * JT:(i + 1) * JT], float(coefs[i]))
                Q = apool.tile([P, 4 * JT], F32)
                nc.vector.tensor_scalar_mul(Q[:], A[:], INV_2PI)
                Kr = apool.tile([P, 4 * JT], F32)
                nc.vector.tensor_scalar(Kr[:], Q[:], MAGIC, MAGIC,
                                        ALU.add, ALU.subtract)
                R = apool.tile([P, 4 * JT], F32)
                nc.vector.cody_waite_cascade(R[:], A[:], Kr[:], CW1, CW2, CW3)
                RC = apool.tile([P, 4 * JT], F32)
                nc.vector.add_range_wrap(RC[:], R[:], HALF_PI, PI_F, TWO_PI_F)

                # SS = [sin | -sin | sin | -sin] slot blocks, C = cos
                SS = apool.tile([P, 4 * D], F32)
                C = apool.tile([P, 4 * JT], F32)
                nc.scalar.activation(SS[:, 0:D], R[:], SIN)
                nc.scalar.activation(C[:], RC[:], SIN)
                nc.vector.tensor_scalar_mul(SS[:, D:2 * D], SS[:, 0:D], -1.0)
                nc.vector.tensor_copy(SS[:, 2 * D:3 * D], SS[:, 0:D])
                nc.vector.tensor_copy(SS[:, 3 * D:4 * D], SS[:, D:2 * D])

                X6 = X[:].rearrange("p (j h s k r) -> p j h s k r",
                                    h=2, s=2, k=2, r=2)
                # innermost rotor applied first: plane index 3, 2, 1, 0
                for i in (3, 2, 1, 0):
                    geom = _PLANE_GEOM[PLANE_BLADES[i]]
                    tbase, sig2, sig3 = taus[i]
                    xa = X6[(slice(None), slice(None)) + geom["a"]]
                    xb = X6[(slice(None), slice(None)) + geom["b"]]
                    c2 = C[:, i * JT:(i + 1) * JT].unsqueeze(2).unsqueeze(3) \
                        .broadcast_to([P, JT, 2, 2])
                    # u1 multiplier: +tau*s2 ; u2 multiplier: -tau*s2
                    s2_u1 = _slot_ap(SS, tbase * D + i * JT, JT, sig2, sig3, D)
                    s2_u2 = _slot_ap(SS, (1 - tbase) * D + i * JT, JT, sig2, sig3, D)
                    t1 = tpool.tile([P, JT, 2, 2], F32, tag="t1")
                    t2 = tpool.tile([P, JT, 2, 2], F32, tag="t2")
                    u1 = tpool.tile([P, JT, 2, 2], F32, tag="u1")
                    u2 = tpool.tile([P, JT, 2, 2], F32, tag="u2")
                    nc.vector.tensor_mul(t1[:], xa, c2)
                    nc.vector.tensor_mul(t2[:], xb, c2)
                    nc.vector.tensor_mul(u1[:], xb, s2_u1)
                    nc.vector.tensor_mul(u2[:], xa, s2_u2)
                    # out_a = c2*xa + tau*s2*xb ; out_b = c2*xb - tau*s2*xa
                    nc.vector.tensor_add(xa, t1[:], u1[:])
                    nc.vector.tensor_add(xb, t2[:], u2[:])

                nc.sync.dma_start(out_d[:, t * JT * MV:(t + 1) * JT * MV], X[:])

    nc.compile()
    return nc


_PROGRAM_CACHE = {}


def _get_program(freqs, coefs, taus):
    key = (tuple(freqs), tuple(coefs), tuple(taus))
    if key not in _PROGRAM_CACHE:
        _PROGRAM_CACHE[key] = _build_program(freqs, coefs, taus)
    return _PROGRAM_CACHE[key]


def kernel(x, pos, bx, by, bz, bw, theta, cayley, biv_mask, scalar_mask):
    x = np.asarray(x, dtype=np.float32)
    pos = np.asarray(pos)
    theta = np.asarray(theta, dtype=np.float32)
    cayley = np.asarray(cayley, dtype=np.float32)

    assert x.shape == (B, L, MV) and pos.shape == (B, L)

    # Scalar plane coefficients c_i = coef[blade_i]; frequencies f_i = row 1
    # of the theta table (theta[p, i] = p * f_i for the RoPE-style schedule —
    # verified below against the full table).
    coefs = [float(np.asarray(c, dtype=np.float32).reshape(MV)[b])
             for c, b in zip((bx, by, bz, bw), PLANE_BLADES)]
    freqs = [float(theta.reshape(MAX_LEN, 4)[1, i]) for i in range(4)]
    th_check = np.arange(MAX_LEN, dtype=np.float32)[:, None] * \
        np.asarray(freqs, dtype=np.float32)[None, :]
    assert np.array_equal(th_check, theta.reshape(MAX_LEN, 4)), \
        "theta table is not linear in position; kernel assumption violated"

    taus = tuple(_derive_tau_pattern(cayley, PLANE_BLADES[i],
                                     _PLANE_GEOM[PLANE_BLADES[i]])
                 for i in range(4))

    nc = _get_program(freqs, coefs, taus)

    pos_i = np.clip(pos, 0, MAX_LEN - 1).astype(np.int32)
    in_maps = []
    for g in range(NCORES):
        xg = np.ascontiguousarray(
            x[g * ROWS_PER_CORE:(g + 1) * ROWS_PER_CORE]).reshape(P, J * MV)
        pg = np.ascontiguousarray(
            pos_i[g * ROWS_PER_CORE:(g + 1) * ROWS_PER_CORE]).reshape(P, J)
        in_maps.append({"x": xg, "pos": pg})

    res = run_bass_kernel_spmd(nc, in_maps, core_ids=list(range(NCORES)))
    out = np.empty((B, L, MV), dtype=np.float32)
    for g in range(NCORES):
        out[g * ROWS_PER_CORE:(g + 1) * ROWS_PER_CORE] = \
            res.results[g]["out"].reshape(ROWS_PER_CORE, L, MV)
    return out
